# revision 2
# baseline (speedup 1.0000x reference)
"""MoE routed linear (nn_L2MLinear): y[b] = x[b] @ W_pool[idx[b]] + B_pool[idx[b]].

Full shapes: x [32, 512, 1024] f32, W_pool [64, 1024, 1024] f32,
B_pool [64, 1024] f32, idx [32, 1] i64, frozen_mask [64] bool (fwd no-op).

Strategy (8 NeuronCores, SPMD):
  - Host: gather per-sample expert weights W_pool[idx] / B_pool[idx] (dispatch),
    transpose x to [IN, N] per sample so the kernel's stationary matmul operand
    loads contiguously, shard batch 4 samples/core.
  - Device (per core): for each sample, y = xT.T @ W + b via PE matmuls
    (fp32 data in float32r streaming mode), bias added during PSUM->SBUF
    eviction on the vector engine.
"""

import os

import numpy as np

_B, _N, _IN, _OUT, _P = 32, 512, 1024, 1024, 64
_NCORES = 8
_BPC = _B // _NCORES  # samples per core
_FREE = 512  # PSUM bank free-dim limit for fp32
_KT = _IN // 128  # contraction tiles
_MT = _N // 128  # token tiles
_NH = _OUT // _FREE  # output column halves

_cache: dict = {}


def _build(mode: str):
    import concourse.bacc as bacc
    import concourse.mybir as mybir
    import concourse.tile as tile

    f32 = mybir.dt.float32
    f32r = mybir.dt.float32r
    bf16 = mybir.dt.bfloat16

    nc = bacc.Bacc("TRN2", target_bir_lowering=False)

    with tile.TileContext(nc) as tc:
        with tc.tile_pool(name="dram", bufs=1, space="DRAM") as dram:
            if mode == "bf16x3":
                xh_d = dram.tile([_BPC, _IN, _N], bf16, kind="ExternalInput", name="xh", uniquify=False)
                xl_d = dram.tile([_BPC, _IN, _N], bf16, kind="ExternalInput", name="xl", uniquify=False)
                wh_d = dram.tile([_BPC, _IN, _OUT], bf16, kind="ExternalInput", name="wh", uniquify=False)
                wl_d = dram.tile([_BPC, _IN, _OUT], bf16, kind="ExternalInput", name="wl", uniquify=False)
            elif mode == "bf16":
                xh_d = dram.tile([_BPC, _IN, _N], bf16, kind="ExternalInput", name="xh", uniquify=False)
                wh_d = dram.tile([_BPC, _IN, _OUT], bf16, kind="ExternalInput", name="wh", uniquify=False)
            else:  # f32r / f32
                xt_d = dram.tile([_BPC, _IN, _N], f32, kind="ExternalInput", name="xt", uniquify=False)
                w_d = dram.tile([_BPC, _IN, _OUT], f32, kind="ExternalInput", name="w", uniquify=False)
            b_d = dram.tile([_BPC, _OUT], f32, kind="ExternalInput", name="bias", uniquify=False)
            y_d = dram.tile([_BPC, _N, _OUT], f32, kind="ExternalOutput", name="y", uniquify=False)

            with (
                tc.tile_pool(name="xp", bufs=2) as xp,
                tc.tile_pool(name="wp", bufs=2) as wp,
                tc.tile_pool(name="bp", bufs=2) as bp,
                tc.tile_pool(name="op", bufs=2) as op_,
                tc.tile_pool(name="pp", bufs=8, space="PSUM") as pp,
            ):
                for s in range(_BPC):
                    if mode == "bf16x3":
                        xh_t = xp.tile([128, _KT, _N], bf16, name="xh_t")
                        nc.sync.dma_start(xh_t, xh_d[s].rearrange("(kt p) n -> p kt n", p=128))
                        xl_t = xp.tile([128, _KT, _N], bf16, name="xl_t")
                        nc.sync.dma_start(xl_t, xl_d[s].rearrange("(kt p) n -> p kt n", p=128))
                        wh_t = wp.tile([128, _KT, _OUT], bf16, name="wh_t")
                        nc.sync.dma_start(wh_t, wh_d[s].rearrange("(kt p) j -> p kt j", p=128))
                        wl_t = wp.tile([128, _KT, _OUT], bf16, name="wl_t")
                        nc.sync.dma_start(wl_t, wl_d[s].rearrange("(kt p) j -> p kt j", p=128))
                    elif mode == "bf16":
                        xh_t = xp.tile([128, _KT, _N], bf16, name="xh_t")
                        nc.sync.dma_start(xh_t, xh_d[s].rearrange("(kt p) n -> p kt n", p=128))
                        wh_t = wp.tile([128, _KT, _OUT], bf16, name="wh_t")
                        nc.sync.dma_start(wh_t, wh_d[s].rearrange("(kt p) j -> p kt j", p=128))
                    else:
                        xt_t = xp.tile([128, _KT, _N], f32, name="xt_t")
                        nc.sync.dma_start(xt_t, xt_d[s].rearrange("(kt p) n -> p kt n", p=128))
                        w_t = wp.tile([128, _KT, _OUT], f32, name="w_t")
                        nc.sync.dma_start(w_t, w_d[s].rearrange("(kt p) j -> p kt j", p=128))

                    bias_t = bp.tile([128, _OUT], f32, name="bias_t")
                    nc.sync.dma_start(bias_t, b_d[s : s + 1, :].partition_broadcast(128))

                    out_t = op_.tile([128, _MT, _OUT], f32, name="out_t")
                    for m in range(_MT):
                        for nh in range(_NH):
                            ps = pp.tile([128, _FREE], f32, name="ps")
                            ms = slice(m * 128, (m + 1) * 128)
                            js = slice(nh * _FREE, (nh + 1) * _FREE)
                            if mode == "bf16x3":
                                for k in range(_KT):
                                    nc.tensor.matmul(ps, xh_t[:, k, ms], wh_t[:, k, js], start=(k == 0), stop=False)
                                for k in range(_KT):
                                    nc.tensor.matmul(ps, xh_t[:, k, ms], wl_t[:, k, js], start=False, stop=False)
                                for k in range(_KT):
                                    nc.tensor.matmul(ps, xl_t[:, k, ms], wh_t[:, k, js], start=False, stop=(k == _KT - 1))
                            elif mode == "bf16":
                                for k in range(_KT):
                                    nc.tensor.matmul(ps, xh_t[:, k, ms], wh_t[:, k, js], start=(k == 0), stop=(k == _KT - 1))
                            else:
                                mmdt = f32r if mode == "f32r" else f32
                                for k in range(_KT):
                                    nc.tensor.matmul(
                                        ps,
                                        xt_t[:, k, ms].bitcast(mmdt),
                                        w_t[:, k, js].bitcast(mmdt),
                                        start=(k == 0),
                                        stop=(k == _KT - 1),
                                    )
                            nc.vector.tensor_tensor(out_t[:, m, js], ps, bias_t[:, js], mybir.AluOpType.add)
                    nc.sync.dma_start(y_d[s].rearrange("(mt p) j -> p mt j", p=128), out_t)

    nc.compile()
    return nc


def _to_bf16_pair(a: np.ndarray):
    import ml_dtypes

    hi = a.astype(ml_dtypes.bfloat16)
    lo = (a - hi.astype(np.float32)).astype(ml_dtypes.bfloat16)
    return hi, lo


def _make_in_maps(inputs, mode):
    x = np.asarray(inputs["x"], dtype=np.float32)
    W_pool = np.asarray(inputs["W_pool"], dtype=np.float32)
    B_pool = np.asarray(inputs["B_pool"], dtype=np.float32)
    i = np.asarray(inputs["idx"]).reshape(_B).astype(np.int64)

    Wg = W_pool[i]  # [B, IN, OUT]
    Bg = B_pool[i]  # [B, OUT]
    xT = np.ascontiguousarray(x.transpose(0, 2, 1))  # [B, IN, N]

    in_maps = []
    for c in range(_NCORES):
        sl = slice(c * _BPC, (c + 1) * _BPC)
        m = {"bias": np.ascontiguousarray(Bg[sl])}
        if mode == "bf16x3":
            xh, xl = _to_bf16_pair(xT[sl])
            wh, wl = _to_bf16_pair(Wg[sl])
            m.update(xh=np.ascontiguousarray(xh), xl=np.ascontiguousarray(xl),
                     wh=np.ascontiguousarray(wh), wl=np.ascontiguousarray(wl))
        elif mode == "bf16":
            import ml_dtypes

            m.update(xh=np.ascontiguousarray(xT[sl].astype(ml_dtypes.bfloat16)),
                     wh=np.ascontiguousarray(Wg[sl].astype(ml_dtypes.bfloat16)))
        else:
            m.update(xt=np.ascontiguousarray(xT[sl]), w=np.ascontiguousarray(Wg[sl]))
        in_maps.append(m)
    return in_maps


def kernel(x, W_pool, B_pool, idx, frozen_mask):
    from concourse.bass_utils import run_bass_kernel_spmd

    mode = os.environ.get("MOE_MODE", "f32r")

    if mode not in _cache:
        _cache[mode] = _build(mode)
    nc = _cache[mode]

    in_maps = _make_in_maps(
        {"x": x, "W_pool": W_pool, "B_pool": B_pool, "idx": idx}, mode
    )
    res = run_bass_kernel_spmd(nc, in_maps, core_ids=list(range(_NCORES)))

    y = np.concatenate([r["y"] for r in res.results], axis=0)
    return np.ascontiguousarray(y.astype(np.float32))


# revision 8
# speedup vs baseline: 541.3552x; 541.3552x over previous
"""MoE routed linear (nn_L2MLinear): y[b] = x[b] @ W_pool[idx[b]] + B_pool[idx[b]].

Full shapes: x [32, 512, 1024] f32, W_pool [64, 1024, 1024] f32,
B_pool [64, 1024] f32, idx [32, 1] i64, frozen_mask [64] bool (fwd no-op).

Strategy (8 NeuronCores, SPMD):
  - Host: gather per-sample expert weights W_pool[idx] / B_pool[idx] (dispatch),
    transpose x to [IN, N] per sample so the kernel's stationary matmul operand
    loads contiguously, shard batch 4 samples/core.
  - Device (per core): for each sample, y = xT.T @ W + b via PE matmuls
    (fp32 data in float32r streaming mode), bias added during PSUM->SBUF
    eviction on the vector engine.
"""

import os

import numpy as np

_B, _N, _IN, _OUT, _P = 32, 512, 1024, 1024, 64
_NCORES = 8
_BPC = _B // _NCORES  # samples per core
_FREE = 512  # PSUM bank free-dim limit for fp32
_KT = _IN // 128  # contraction tiles
_MT = _N // 128  # token tiles
_NH = _OUT // _FREE  # output column halves

_cache: dict = {}


def _build(mode: str):
    import concourse.bacc as bacc
    import concourse.mybir as mybir
    import concourse.tile as tile

    f32 = mybir.dt.float32
    f32r = mybir.dt.float32r
    bf16 = mybir.dt.bfloat16

    nc = bacc.Bacc("TRN2", target_bir_lowering=False)

    with tile.TileContext(nc) as tc:
        with tc.tile_pool(name="dram", bufs=1, space="DRAM") as dram:
            if mode == "bf16x3":
                xh_d = dram.tile([_BPC, _IN, _N], bf16, kind="ExternalInput", name="xh", uniquify=False)
                xl_d = dram.tile([_BPC, _IN, _N], bf16, kind="ExternalInput", name="xl", uniquify=False)
                wh_d = dram.tile([_BPC, _IN, _OUT], bf16, kind="ExternalInput", name="wh", uniquify=False)
                wl_d = dram.tile([_BPC, _IN, _OUT], bf16, kind="ExternalInput", name="wl", uniquify=False)
            elif mode == "bf16":
                xh_d = dram.tile([_BPC, _IN, _N], bf16, kind="ExternalInput", name="xh", uniquify=False)
                wh_d = dram.tile([_BPC, _IN, _OUT], bf16, kind="ExternalInput", name="wh", uniquify=False)
            else:  # f32r / f32
                xt_d = dram.tile([_BPC, _IN, _N], f32, kind="ExternalInput", name="xt", uniquify=False)
                w_d = dram.tile([_BPC, _IN, _OUT], f32, kind="ExternalInput", name="w", uniquify=False)
            b_d = dram.tile([_BPC, _OUT], f32, kind="ExternalInput", name="bias", uniquify=False)
            y_d = dram.tile([_BPC, _N, _OUT], f32, kind="ExternalOutput", name="y", uniquify=False)

            with (
                tc.tile_pool(name="xp", bufs=2) as xp,
                tc.tile_pool(name="wp", bufs=2) as wp,
                tc.tile_pool(name="bp", bufs=2) as bp,
                tc.tile_pool(name="op", bufs=2) as op_,
                tc.tile_pool(name="pp", bufs=8, space="PSUM") as pp,
            ):
                for s in range(_BPC):
                    if mode == "bf16x3":
                        # Chunk DMAs per contraction slice, hi-term data first,
                        # so the first matmul starts after one (xh,wh) k-slice
                        # lands instead of after the full 6MB sample load.
                        xh_t = xp.tile([128, _KT, _N], bf16, name="xh_t")
                        wh_t = wp.tile([128, _KT, _OUT], bf16, name="wh_t")
                        xh_src = xh_d[s].rearrange("(kt p) n -> p kt n", p=128)
                        wh_src = wh_d[s].rearrange("(kt p) j -> p kt j", p=128)
                        for k in range(_KT):
                            nc.sync.dma_start(xh_t[:, k], xh_src[:, k])
                            nc.sync.dma_start(wh_t[:, k], wh_src[:, k])
                        xl_t = xp.tile([128, _KT, _N], bf16, name="xl_t")
                        wl_t = wp.tile([128, _KT, _OUT], bf16, name="wl_t")
                        xl_src = xl_d[s].rearrange("(kt p) n -> p kt n", p=128)
                        wl_src = wl_d[s].rearrange("(kt p) j -> p kt j", p=128)
                        for k in range(_KT):
                            nc.sync.dma_start(wl_t[:, k], wl_src[:, k])
                        for k in range(_KT):
                            nc.sync.dma_start(xl_t[:, k], xl_src[:, k])
                    elif mode == "bf16":
                        xh_t = xp.tile([128, _KT, _N], bf16, name="xh_t")
                        nc.sync.dma_start(xh_t, xh_d[s].rearrange("(kt p) n -> p kt n", p=128))
                        wh_t = wp.tile([128, _KT, _OUT], bf16, name="wh_t")
                        nc.sync.dma_start(wh_t, wh_d[s].rearrange("(kt p) j -> p kt j", p=128))
                    else:
                        xt_t = xp.tile([128, _KT, _N], f32, name="xt_t")
                        nc.sync.dma_start(xt_t, xt_d[s].rearrange("(kt p) n -> p kt n", p=128))
                        w_t = wp.tile([128, _KT, _OUT], f32, name="w_t")
                        nc.sync.dma_start(w_t, w_d[s].rearrange("(kt p) j -> p kt j", p=128))

                    bias_t = bp.tile([128, _OUT], f32, name="bias_t")
                    nc.sync.dma_start(bias_t, b_d[s : s + 1, :].partition_broadcast(128))

                    out_t = op_.tile([128, _MT, _OUT], f32, name="out_t")
                    y_dst = y_d[s].rearrange("(mt p) j -> p mt j", p=128)
                    if mode == "bf16x3":
                        # k-outer sweep over all 8 PSUM banks per term: each
                        # k-step only needs that k's DMA chunk, so PE streams
                        # at DMA arrival rate instead of blocking per-bank on
                        # the full sample load.
                        ps_l = [pp.tile([128, _FREE], f32, name="ps") for _ in range(_MT * _NH)]
                        terms = ((xh_t, wh_t), (xh_t, wl_t), (xl_t, wh_t))
                        for t, (xa, wa) in enumerate(terms):
                            for k in range(_KT):
                                for m in range(_MT):
                                    ms = slice(m * 128, (m + 1) * 128)
                                    for nh in range(_NH):
                                        js = slice(nh * _FREE, (nh + 1) * _FREE)
                                        nc.tensor.matmul(
                                            ps_l[m * _NH + nh],
                                            xa[:, k, ms],
                                            wa[:, k, js],
                                            start=(t == 0 and k == 0),
                                            stop=(t == 2 and k == _KT - 1),
                                        )
                        for m in range(_MT):
                            for nh in range(_NH):
                                js = slice(nh * _FREE, (nh + 1) * _FREE)
                                nc.vector.tensor_tensor(
                                    out_t[:, m, js], ps_l[m * _NH + nh], bias_t[:, js], mybir.AluOpType.add
                                )
                            nc.sync.dma_start(y_dst[:, m], out_t[:, m])
                    else:
                        for m in range(_MT):
                            for nh in range(_NH):
                                ps = pp.tile([128, _FREE], f32, name="ps")
                                ms = slice(m * 128, (m + 1) * 128)
                                js = slice(nh * _FREE, (nh + 1) * _FREE)
                                if mode == "bf16":
                                    for k in range(_KT):
                                        nc.tensor.matmul(ps, xh_t[:, k, ms], wh_t[:, k, js], start=(k == 0), stop=(k == _KT - 1))
                                else:
                                    mmdt = f32r if mode == "f32r" else f32
                                    for k in range(_KT):
                                        nc.tensor.matmul(
                                            ps,
                                            xt_t[:, k, ms].bitcast(mmdt),
                                            w_t[:, k, js].bitcast(mmdt),
                                            start=(k == 0),
                                            stop=(k == _KT - 1),
                                        )
                                nc.vector.tensor_tensor(out_t[:, m, js], ps, bias_t[:, js], mybir.AluOpType.add)
                            nc.sync.dma_start(y_dst[:, m], out_t[:, m])

    nc.compile()
    return nc


def _to_bf16_pair(a: np.ndarray):
    import ml_dtypes

    hi = a.astype(ml_dtypes.bfloat16)
    lo = (a - hi.astype(np.float32)).astype(ml_dtypes.bfloat16)
    return hi, lo


def _make_in_maps(inputs, mode):
    x = np.asarray(inputs["x"], dtype=np.float32)
    W_pool = np.asarray(inputs["W_pool"], dtype=np.float32)
    B_pool = np.asarray(inputs["B_pool"], dtype=np.float32)
    i = np.asarray(inputs["idx"]).reshape(_B).astype(np.int64)

    Wg = W_pool[i]  # [B, IN, OUT]
    Bg = B_pool[i]  # [B, OUT]
    xT = np.ascontiguousarray(x.transpose(0, 2, 1))  # [B, IN, N]

    in_maps = []
    for c in range(_NCORES):
        sl = slice(c * _BPC, (c + 1) * _BPC)
        m = {"bias": np.ascontiguousarray(Bg[sl])}
        if mode == "bf16x3":
            xh, xl = _to_bf16_pair(xT[sl])
            wh, wl = _to_bf16_pair(Wg[sl])
            m.update(xh=np.ascontiguousarray(xh), xl=np.ascontiguousarray(xl),
                     wh=np.ascontiguousarray(wh), wl=np.ascontiguousarray(wl))
        elif mode == "bf16":
            import ml_dtypes

            m.update(xh=np.ascontiguousarray(xT[sl].astype(ml_dtypes.bfloat16)),
                     wh=np.ascontiguousarray(Wg[sl].astype(ml_dtypes.bfloat16)))
        else:
            m.update(xt=np.ascontiguousarray(xT[sl]), w=np.ascontiguousarray(Wg[sl]))
        in_maps.append(m)
    return in_maps


def kernel(x, W_pool, B_pool, idx, frozen_mask):
    from concourse.bass_utils import run_bass_kernel_spmd

    mode = os.environ.get("MOE_MODE", "bf16x3")

    if mode not in _cache:
        _cache[mode] = _build(mode)
    nc = _cache[mode]

    in_maps = _make_in_maps(
        {"x": x, "W_pool": W_pool, "B_pool": B_pool, "idx": idx}, mode
    )
    res = run_bass_kernel_spmd(nc, in_maps, core_ids=list(range(_NCORES)))

    y = np.concatenate([r["y"] for r in res.results], axis=0)
    return np.ascontiguousarray(y.astype(np.float32))
